# revision 1
# baseline (speedup 1.0000x reference)
"""Trainium2 Bass kernel for nn_ConvertParamsToMomentsLayer.

Full-input contract: kernel(**inputs) takes the complete (unsharded) arrays
  wt[256, 256, 512], varh_diag[256, 256], sig2[256], b[256, 512], muh[256, 256]
and returns (mu[256, 768], var[256, 768, 768]) as float32, matching

  varvh = varh_diag[:, :, None] * wt
  varv  = wt.T @ varvh + sig2 * I
  muv   = b + wt.T @ muh
  mu    = concat([muv, muh])
  var   = [[varv, varvh.T], [varvh, diag(varh_diag)]]

Strategy: pure batch parallelism — the 256 batch elements are split 32-per-core
across 8 NeuronCores (SPMD, one NEFF, per-core input slices). Per batch
element, one fp32 matmul chain computes the upper-triangular 128-row tiles of
the symmetric varv (lower tiles come from PE transposes of the upper ones),
varvh is formed in SBUF by a per-partition scalar multiply and doubles as the
matmul moving operand, and the full 768x768 block matrix is assembled in SBUF
and streamed to HBM. The kernel is DMA-bound (~92 MB of HBM traffic per core);
loads run on the ACT HWDGE ring and stores on the SP ring so they overlap, and
each var store is split into three pieces so writeback starts before the
matmuls finish.
"""

from contextlib import ExitStack

import numpy as np

import concourse.bass as bass
import concourse.mybir as mybir
import concourse.tile as tile
from concourse import bacc
from concourse.bass_utils import run_bass_kernel_spmd
from concourse.masks import make_identity

F32 = mybir.dt.float32

B = 256
NH = 256
NV = 512
NT = NV + NH  # 768
P = 128
KC = NH // P  # 2 contraction chunks of 128
MT = NV // P  # 4 row tiles of varv
NCORES = 8
NB = B // NCORES  # 32 batch elements per core


def _build_kernel():
    nc = bacc.Bacc("TRN2", target_bir_lowering=False, debug=False)

    wt_d = nc.dram_tensor("wt", [NB, NH, NV], F32, kind="ExternalInput").ap()
    varh_d = nc.dram_tensor("varh_diag", [NB, NH], F32, kind="ExternalInput").ap()
    sig2_d = nc.dram_tensor("sig2", [NB], F32, kind="ExternalInput").ap()
    b_d = nc.dram_tensor("b", [NB, NV], F32, kind="ExternalInput").ap()
    muh_d = nc.dram_tensor("muh", [NB, NH], F32, kind="ExternalInput").ap()

    mu_d = nc.dram_tensor("mu", [NB, NT], F32, kind="ExternalOutput").ap()
    var_d = nc.dram_tensor("var", [NB, NT, NT], F32, kind="ExternalOutput").ap()

    with tile.TileContext(nc) as tc, ExitStack() as ctx:
        consts = ctx.enter_context(tc.tile_pool(name="consts", bufs=1))
        wt_pool = ctx.enter_context(tc.tile_pool(name="wt", bufs=3))
        asm_pool = ctx.enter_context(tc.tile_pool(name="asm", bufs=3))
        pv_pool = ctx.enter_context(tc.tile_pool(name="pv", bufs=3, space="PSUM"))
        pt_pool = ctx.enter_context(tc.tile_pool(name="pt", bufs=3, space="PSUM"))
        pmu_pool = ctx.enter_context(tc.tile_pool(name="pmu", bufs=1, space="PSUM"))

        eye = consts.tile([P, P], F32, tag="eye")
        make_identity(nc, eye)

        # per-partition scalar columns: x_cols[p, i, k] = x[i, k*128 + p]
        varh_cols = consts.tile([P, NB, KC], F32, tag="varh_cols")
        nc.gpsimd.dma_start(out=varh_cols, in_=varh_d.rearrange("i (k p) -> p i k", p=P))
        muh_cols = consts.tile([P, NB, KC], F32, tag="muh_cols")
        nc.gpsimd.dma_start(out=muh_cols, in_=muh_d.rearrange("i (k p) -> p i k", p=P))
        sig2_row = consts.tile([P, NB], F32, tag="sig2_row")
        nc.gpsimd.dma_start(out=sig2_row, in_=sig2_d.partition_broadcast(P))

        mu_t = consts.tile([NB, NT], F32, tag="mu_t")
        nc.sync.dma_start(out=mu_t[:, 0:NV], in_=b_d)
        nc.sync.dma_start(out=mu_t[:, NV:NT], in_=muh_d)

        # one PSUM bank collecting every (i, m, k) single-column muv matmul
        pmu = pmu_pool.tile([P, MT * KC * NB], F32, tag="pmu")

        def cp(eng_is_scalar, out_ap, in_ap):
            if eng_is_scalar:
                nc.scalar.copy(out_ap, in_ap)
            else:
                nc.vector.tensor_copy(out_ap, in_ap)

        for i in range(NB):
            wt_t = wt_pool.tile([P, KC, NV], F32, tag="wt_t")
            nc.scalar.dma_start(out=wt_t, in_=wt_d[i].rearrange("(k p) c -> p k c", p=P))

            # var[i] assembly: row tile t holds rows t*128..t*128+127 of var[i]
            # t 0..3: [varv_m | varvh.T_m];  t 4..5: [varvh_k | diag block]
            asm = asm_pool.tile([P, 6, NT], F32, tag="asm")

            for k in range(KC):
                nc.vector.tensor_scalar_mul(
                    asm[:, 4 + k, 0:NV], wt_t[:, k, :], varh_cols[:, i, k : k + 1]
                )
                nc.gpsimd.memset(asm[:, 4 + k, NV + P * (1 - k) : NV + P * (2 - k)], 0.0)
                nc.vector.tensor_scalar_mul(
                    asm[:, 4 + k, NV + P * k : NV + P * (k + 1)],
                    eye,
                    varh_cols[:, i, k : k + 1],
                )

            for m in range(MT):
                lo = m * P  # varv is symmetric: matmul only cols >= m*128
                vv = pv_pool.tile([P, NV], F32, tag="vv")
                for k in range(KC):
                    nc.tensor.matmul(
                        vv[:, lo:NV],
                        wt_t[:, k, m * P : (m + 1) * P],
                        asm[:, 4 + k, lo:NV],
                        start=(k == 0),
                        stop=(k == KC - 1),
                    )
                    col = m * (2 * NB) + i * 2 + k
                    nc.tensor.matmul(
                        pmu[:, col : col + 1],
                        wt_t[:, k, m * P : (m + 1) * P],
                        muh_cols[:, i, k : k + 1],
                        start=True,
                        stop=True,
                    )

                if m % 2 == 0:
                    vt = pt_pool.tile([P, NV], F32, tag="vt")
                for k in range(KC):
                    nc.tensor.transpose(
                        vt[:, (m % 2) * 2 * P + k * P : (m % 2) * 2 * P + (k + 1) * P],
                        asm[:, 4 + k, m * P : (m + 1) * P],
                        eye,
                    )

                if m > 0:
                    # lower-triangle varv blocks (m, n<m) = transpose of (n, m),
                    # already in SBUF at asm[:, n, m*128:(m+1)*128]
                    sym = pt_pool.tile([P, (MT - 1) * P], F32, tag="vt")
                    for n in range(m):
                        nc.tensor.transpose(
                            sym[:, n * P : (n + 1) * P],
                            asm[:, n, m * P : (m + 1) * P],
                            eye,
                        )
                    for n in range(m):
                        cp(
                            (m + n) % 2 == 0,
                            asm[:, m, n * P : (n + 1) * P],
                            sym[:, n * P : (n + 1) * P],
                        )

                if m < MT - 1:
                    cp(m % 2 == 0, asm[:, m, (m + 1) * P : NV], vv[:, (m + 1) * P : NV])
                nc.vector.scalar_tensor_tensor(
                    out=asm[:, m, m * P : (m + 1) * P],
                    in0=eye,
                    scalar=sig2_row[:, i : i + 1],
                    in1=vv[:, m * P : (m + 1) * P],
                    op0=mybir.AluOpType.mult,
                    op1=mybir.AluOpType.add,
                )
                cp(
                    m % 2 != 0,
                    asm[:, m, NV:NT],
                    vt[:, (m % 2) * 2 * P : (m % 2) * 2 * P + 2 * P],
                )

            var_i = var_d[i].rearrange("(t p) c -> p t c", p=P)
            nc.sync.dma_start(out=var_i[:, 4:6, :], in_=asm[:, 4:6, :])
            nc.sync.dma_start(out=var_i[:, 0:2, :], in_=asm[:, 0:2, :])
            nc.sync.dma_start(out=var_i[:, 2:4, :], in_=asm[:, 2:4, :])

        # mu epilogue: muv columns -> rows, add b, store
        muv_cols = consts.tile([P, MT * NB], F32, tag="muv_cols")
        pmu_s = consts.tile([P, MT * KC * NB], F32, tag="pmu_s")
        nc.vector.tensor_copy(pmu_s, pmu)
        pmu3 = pmu_s.rearrange("p (c two) -> p c two", two=2)
        nc.vector.tensor_add(muv_cols, pmu3[:, :, 0], pmu3[:, :, 1])

        pmut = pmu_pool.tile([NB, MT * P], F32, tag="pmut")
        for m in range(MT):
            nc.tensor.transpose(
                pmut[:, m * P : (m + 1) * P], muv_cols[:, m * NB : (m + 1) * NB], eye
            )
        nc.vector.tensor_add(mu_t[:, 0:NV], mu_t[:, 0:NV], pmut[0:NB, :])
        nc.sync.dma_start(out=mu_d, in_=mu_t)

    nc.compile()
    return nc


_NC_CACHE = None


def kernel(wt, varh_diag, sig2, b, muh):
    global _NC_CACHE
    if _NC_CACHE is None:
        _NC_CACHE = _build_kernel()
    nc = _NC_CACHE

    wt = np.ascontiguousarray(np.asarray(wt, dtype=np.float32))
    varh_diag = np.ascontiguousarray(np.asarray(varh_diag, dtype=np.float32))
    sig2 = np.ascontiguousarray(np.asarray(sig2, dtype=np.float32))
    b = np.ascontiguousarray(np.asarray(b, dtype=np.float32))
    muh = np.ascontiguousarray(np.asarray(muh, dtype=np.float32))

    in_maps = []
    for c in range(NCORES):
        s = slice(c * NB, (c + 1) * NB)
        in_maps.append(
            {
                "wt": wt[s],
                "varh_diag": varh_diag[s],
                "sig2": sig2[s],
                "b": b[s],
                "muh": muh[s],
            }
        )

    res = run_bass_kernel_spmd(nc, in_maps, core_ids=list(range(NCORES)))
    mu = np.concatenate([r["mu"] for r in res.results], axis=0)
    var = np.concatenate([r["var"] for r in res.results], axis=0)
    return mu, var
